# revision 1
# baseline (speedup 1.0000x reference)
"""Causal self-attention (B=4, T=2048, C=1024, 16 heads x 64) on 8 TRN2 cores.

Sharding: core c handles batch b = c//2 and head-group g = c%2 (8 heads).
Each core computes a partial output  y_part = attn_heads(x[b]) @ w_proj[rows]
and the host sums the two partials per batch (Megatron row-parallel proj).

On-core pipeline (matmuls in float32r: full PE rate, ~1e-4 accuracy):
  - x[b] is fed pre-transposed (xT, [C,T]) so every matmul contracts over
    the partition dim.
  - QT/KT = w[:,cols]^T @ x^T  ([128=2 heads*64, T], head-major partitions)
  - V     = x @ wv             ([T, 512], natural layout) + ones column
  - S^T   = K_blk @ Q^T        ([128 k, 512 q] tiles, causal blocks only;
                                the two heads run on disjoint PE row groups)
  - P^T   = exp(S^T/8) with a triangular additive mask on diagonal blocks
  - y^T   = [V|1]^T @ P^T      (accumulated over k blocks; row 64 = softmax
                                denominator, fused via the ones column)
  - normalize with DVE reciprocal + K=1 ones-matmul partition broadcast,
    writing bf16 Y^T straight into the projection's stationary layout
  - out   = Y @ wp             (bf16 stationary/moving, fp32 accumulate)
"""
from contextlib import ExitStack

import numpy as np

import concourse.mybir as mybir
import concourse.tile as tile
from concourse import bacc
from concourse.bass_utils import run_bass_kernel_spmd

dt = mybir.dt
AF = mybir.ActivationFunctionType

T = 2048
C = 1024
HD = 64
TQ = 512            # q-tile width
NQT = T // TQ       # 4
NKB = T // 128      # 16 k-blocks
GK = 2              # k-blocks per exp group
SCALE = 1.0 / 8.0   # 1/sqrt(64)
NEG = -1e30
LOOP_R = 1          # >1: repeat whole kernel on-device (timing builds only)


def build():
    nc = bacc.Bacc(target_bir_lowering=False, debug=False, dynamic_dma_scratch_size=2048)
    f32, f32r, bf16 = dt.float32, dt.float32r, dt.bfloat16

    xT_d = nc.dram_tensor("xT", [C, T], f32r, kind="ExternalInput")
    wq_d = nc.dram_tensor("wq", [C, 512], f32r, kind="ExternalInput")
    wk_d = nc.dram_tensor("wk", [C, 512], f32r, kind="ExternalInput")
    wv_d = nc.dram_tensor("wv", [C, 512], f32r, kind="ExternalInput")
    wp_d = nc.dram_tensor("wp", [512, C], f32r, kind="ExternalInput")
    onesr_d = nc.dram_tensor("onesr", [128, 64], f32r, kind="ExternalInput")
    out_d = nc.dram_tensor("out", [T, C], f32, kind="ExternalOutput")

    with tile.TileContext(nc) as tc, ExitStack() as ctx:
        if LOOP_R > 1:
            E = mybir.EngineType
            ctx.enter_context(tc.For_i(0, LOOP_R, 1, hint_engines=(
                E.PE, E.DVE, E.Activation, E.SP, E.Pool)))
        cp = ctx.enter_context(tc.tile_pool(name="consts", bufs=1))

        # Y^T in projection-stationary layout + V with fused ones columns
        YTL = cp.tile([128, 4 * T], f32r, tag="ytl")
        VA = cp.tile([128, NKB * 520], f32r, tag="va")

        QTs, KTs = [], []
        qkt = ctx.enter_context(tc.tile_pool(name="qkt", bufs=1))
        with tc.tile_pool(name="psA", bufs=4, space="PSUM") as psA:
            with tc.tile_pool(name="pxt", bufs=1) as pxt:
                XT = pxt.tile([128, 8 * T], f32r, tag="xt")
                with tc.tile_pool(name="pwq", bufs=1) as pwq:
                    WQ = pwq.tile([128, 8 * 512], f32r, tag="wq")
                    for k in range(8):
                        nc.sync.dma_start(out=XT[:, 2048 * k: 2048 * (k + 1)],
                                          in_=xT_d.ap()[128 * k: 128 * (k + 1), :])
                        nc.sync.dma_start(out=WQ[:, 512 * k: 512 * (k + 1)],
                                          in_=wq_d.ap()[128 * k: 128 * (k + 1), :])
                    for m in range(4):
                        QT = qkt.tile([128, T], f32r, tag=f"qt{m}", name=f"qt{m}")
                        QTs.append(QT)
                        for tt in range(NQT):
                            pmm = psA.tile([128, 512], f32, tag="psmm", name="pmm")
                            for k in range(8):
                                nc.tensor.matmul(
                                    pmm[:, :],
                                    lhsT=WQ[:, 512 * k + 128 * m: 512 * k + 128 * m + 128],
                                    rhs=XT[:, 2048 * k + 512 * tt: 2048 * k + 512 * tt + 512],
                                    start=(k == 0), stop=(k == 7),
                                )
                            nc.vector.tensor_copy(QT[:, 512 * tt: 512 * tt + 512], pmm[:, :])
                with tc.tile_pool(name="pwk", bufs=1) as pwk:
                    WK = pwk.tile([128, 8 * 512], f32r, tag="wk")
                    nc.sync.dma_start(
                        out=WK[:, :].rearrange("p (n t) -> p n t", n=8),
                        in_=wk_d.ap().rearrange("(n p) t -> p n t", p=128))
                    for m in range(4):
                        KT = qkt.tile([128, T], f32r, tag=f"kt{m}", name=f"kt{m}")
                        KTs.append(KT)
                        for tt in range(NQT):
                            pmm = psA.tile([128, 512], f32, tag="psmm", name="pmm")
                            for k in range(8):
                                nc.tensor.matmul(
                                    pmm[:, :],
                                    lhsT=WK[:, 512 * k + 128 * m: 512 * k + 128 * m + 128],
                                    rhs=XT[:, 2048 * k + 512 * tt: 2048 * k + 512 * tt + 512],
                                    start=(k == 0), stop=(k == 7),
                                )
                            nc.vector.tensor_copy(KT[:, 512 * tt: 512 * tt + 512], pmm[:, :])
                with tc.tile_pool(name="pwv", bufs=1) as pwv:
                    WV = pwv.tile([128, 8 * 512], f32r, tag="wv")
                    ONES = pwv.tile([128, 128], f32, tag="ones")
                    nc.gpsimd.memset(ONES[:, :], 1.0)
                    nc.sync.dma_start(
                        out=WV[:, :].rearrange("p (n t) -> p n t", n=8),
                        in_=wv_d.ap().rearrange("(n p) t -> p n t", p=128))
                    nc.vector.tensor_copy(
                        VA[:, :].rearrange("p (k h e) -> p k h e", k=NKB, h=8)[:, :, :, 64:65],
                        ONES[:, :].rearrange("p (k h e) -> p k h e", k=NKB, h=8))
                    for kb in range(NKB):
                        psv = psA.tile([128, 512], f32, tag="psmm", name="psv")
                        for k in range(8):
                            nc.tensor.matmul(
                                psv[:, :],
                                lhsT=XT[:, 2048 * k + 128 * kb: 2048 * k + 128 * kb + 128],
                                rhs=WV[:, 512 * k: 512 * k + 512],
                                start=(k == 0), stop=(k == 7),
                            )
                        nc.vector.tensor_copy(
                            VA[:, 520 * kb: 520 * kb + 520]
                            .rearrange("p (h e) -> p h e", h=8)[:, :, 0:64],
                            psv[:, :].rearrange("p (h e) -> p h e", h=8))

        # ---- attention: deep-pipelined score groups shared across heads
        with tc.tile_pool(name="pt", bufs=4) as ptp, \
             tc.tile_pool(name="sm", bufs=1) as sm, \
             tc.tile_pool(name="psS", bufs=3, space="PSUM") as psS, \
             tc.tile_pool(name="psY", bufs=1, space="PSUM") as psY:
            TRI = sm.tile([128, 128], f32, tag="tri")
            nc.gpsimd.memset(TRI[:, :], 0.0)
            nc.gpsimd.affine_select(
                out=TRI[:, :], in_=TRI[:, :],
                compare_op=mybir.AluOpType.is_ge, fill=NEG,
                base=0, pattern=[[1, 128]], channel_multiplier=-1,
            )
            ONESR = sm.tile([128, 64], f32r, tag="onesr")
            nc.sync.dma_start(out=ONESR[:, :], in_=onesr_d.ap())
            for m in range(4):
                QT, KT = QTs[m], KTs[m]
                for qi in range(NQT):
                    nkb = 4 * qi + 4
                    psy = [psY.tile([65, 512], f32, tag=f"psy{a}", name=f"psy{a}")
                           for a in (0, 1)]
                    for g0 in range(0, nkb, GK):
                        grp = range(g0, min(g0 + GK, nkb))
                        for a in (0, 1):
                            pss = psS.tile([128, GK * 512], f32, tag="pss", name="pss")
                            for j, kb in enumerate(grp):
                                nc.tensor.matmul(
                                    pss[:, 512 * j: 512 * j + 512],
                                    lhsT=KT[64 * a: 64 * a + 64, 128 * kb: 128 * kb + 128],
                                    rhs=QT[64 * a: 64 * a + 64, TQ * qi: TQ * qi + 512],
                                    start=True, stop=True,
                                )
                            for j, kb in enumerate(grp):
                                r = kb - 4 * qi
                                if r >= 0:
                                    sl = slice(512 * j + 128 * r, 512 * j + 128 * r + 128)
                                    nc.vector.tensor_add(pss[:, sl], pss[:, sl], TRI[:, :])
                            PT = ptp.tile([128, GK * 512], f32r, tag="pt")
                            w = 512 * len(grp)
                            nc.scalar.activation(PT[:, :w], pss[:, :w], AF.Exp, scale=SCALE)
                            for j, kb in enumerate(grp):
                                r = kb - 4 * qi
                                c0 = 128 * r if r >= 0 else 0
                                nc.tensor.matmul(
                                    psy[a][:, c0:512],
                                    lhsT=VA[:, 520 * kb + 65 * (2 * m + a):
                                            520 * kb + 65 * (2 * m + a) + 65],
                                    rhs=PT[:, 512 * j + c0: 512 * j + 512],
                                    start=(kb == 0), stop=(kb == nkb - 1),
                                )
                    for a in (0, 1):
                        rb = sm.tile([128, 512], f32r, tag="rb", bufs=2)
                        with nc.allow_low_precision(reason="f32r is fp32-width"):
                            nc.vector.reciprocal(rb[64:65, :], psy[a][64:65, :])
                        # broadcast along partitions via a K=1 ones-matmul
                        BC = psS.tile([64, 512], f32, tag="pss", name="bc")
                        nc.tensor.matmul(BC[:, :], lhsT=ONESR[64:65, 0:64],
                                         rhs=rb[64:65, :], start=True, stop=True)
                        nc.vector.tensor_copy(rb[0:64, :], BC[:, :])
                        if a == 0:
                            nc.vector.tensor_mul(
                                YTL[0:64, 2048 * m + TQ * qi: 2048 * m + TQ * qi + 512],
                                psy[a][0:64, :], rb[0:64, :])
                        else:
                            YTT = sm.tile([64, 512], f32r, tag="ytt", bufs=2)
                            nc.vector.tensor_mul(YTT[:, :], psy[a][0:64, :], rb[0:64, :])
                            nc.sync.dma_start(
                                out=YTL[64:128, 2048 * m + TQ * qi: 2048 * m + TQ * qi + 512],
                                in_=YTT[:, :])

        # ---- output projection: out[t, c] = sum_d Y^T[d, t] * wp[d, c]
        with tc.tile_pool(name="ob", bufs=4) as obp, \
             tc.tile_pool(name="psC", bufs=4, space="PSUM") as psC:
            WP = obp.tile([128, 4 * C], f32r, tag="wp")
            nc.sync.dma_start(
                out=WP[:, :].rearrange("p (n t) -> p n t", n=4),
                in_=wp_d.ap().rearrange("(n p) t -> p n t", p=128))
            for t in range(16):
                for h in range(2):
                    pso = psC.tile([128, 512], f32, tag="pso")
                    for p in range(4):
                        nc.tensor.matmul(
                            pso[:, :],
                            lhsT=YTL[:, 2048 * p + 128 * t: 2048 * p + 128 * t + 128],
                            rhs=WP[:, 1024 * p + 512 * h: 1024 * p + 512 * h + 512],
                            start=(p == 0), stop=(p == 3),
                        )
                    ob = obp.tile([128, 512], f32, tag="ob")
                    nc.vector.tensor_copy(ob[:, :], pso[:, :])
                    nc.sync.dma_start(
                        out=out_d.ap()[128 * t: 128 * t + 128, 512 * h: 512 * h + 512],
                        in_=ob[:, :])
    return nc


def make_in_maps(x, w_attn, w_proj):
    x = np.asarray(x, dtype=np.float32)
    w_attn = np.asarray(w_attn, dtype=np.float32)
    w_proj = np.asarray(w_proj, dtype=np.float32)
    in_maps = []
    for c in range(8):
        b, g = divmod(c, 2)
        in_maps.append({
            "xT": np.ascontiguousarray(x[b].T),
            "wq": np.ascontiguousarray(w_attn[:, 512 * g: 512 * (g + 1)]),
            "wk": np.ascontiguousarray(w_attn[:, 1024 + 512 * g: 1024 + 512 * (g + 1)]),
            "wv": np.ascontiguousarray(w_attn[:, 2048 + 512 * g: 2048 + 512 * (g + 1)]),
            "wp": np.ascontiguousarray(w_proj[512 * g: 512 * (g + 1), :]),
            "onesr": np.ones((128, 64), np.float32),
        })
    return in_maps


_nc_cache = None


def kernel(x, w_attn, w_proj):
    global _nc_cache
    if _nc_cache is None:
        nc = build()
        nc.compile()
        _nc_cache = nc
    nc = _nc_cache
    res = run_bass_kernel_spmd(nc, make_in_maps(x, w_attn, w_proj), list(range(8)))
    outs = [res.results[c]["out"] for c in range(8)]
    y = np.empty((4, T, C), np.float32)
    for b in range(4):
        y[b] = outs[2 * b] + outs[2 * b + 1]
    return y

